# revision 11
# baseline (speedup 1.0000x reference)
"""Balanced dice loss (histogram binning) on 8 Trainium2 NeuronCores.

Math: with t ∈ {0,1} and p = sigmoid(x), the reference loss only needs
four global sums:
    S_t   = Σ t            (count of ones — the bincount)
    S_pt  = Σ p·t
    S_pp  = Σ p²
    S_ppt = Σ p²·t
Then with c1 = S_t, c0 = N − c1, w0 = 1/(c0+s)², w1 = 1/(c1+s)²:
    intersection = w1·S_pt
    denominator  = w0·(S_pp − S_ppt) + w1·(S_ppt + c1)
    dice = 1 − (2·I + s)/(D + s)

Device kernel (data-parallel over 8 cores, batch-sharded), per [128,F] tile:
    ACT : p = sigmoid(x); sq = p² (+row-accum → S_pp); float(t) (+accum → S_t)
    DVE : u = p·t, w = sq·t  (tensor_tensor with int32 in1, f32 out)
    PE  : ones[128,1]ᵀ @ u/w chunks → PSUM column-sum accumulation (S_pt, S_ppt)
Per-partition/per-tile partials are DMA'd out; host reduces in float64.
"""

import numpy as np

import concourse.bacc as bacc
import concourse.mybir as mybir
from concourse.bass_utils import run_bass_kernel_spmd
from concourse.tile import TileContext

N_CORES = 8
P = 128
TOTAL = 32 * 1024 * 1024  # elements in the full problem
PER_CORE = TOTAL // N_CORES  # 4,194,304
FREE = PER_CORE // P  # 32,768 f32 per partition
F = 4096  # tile free-dim
NT = FREE // F  # tiles per core
MMN = 512  # matmul moving free-dim (one PSUM bank)
NCH = F // MMN  # matmul chunks per tile
SMOOTH = 1e-05

_nc_cache = None


def _build_bass():
    nc = bacc.Bacc(None, target_bir_lowering=False)
    x = nc.dram_tensor("input", [P, FREE], mybir.dt.float32, kind="ExternalInput")
    t = nc.dram_tensor("target", [P, FREE], mybir.dt.int32, kind="ExternalInput")
    o_pt = nc.dram_tensor("o_pt", [1, MMN], mybir.dt.float32, kind="ExternalOutput")
    o_ppt = nc.dram_tensor("o_ppt", [1, MMN], mybir.dt.float32, kind="ExternalOutput")
    o_pp = nc.dram_tensor("o_pp", [P, NT], mybir.dt.float32, kind="ExternalOutput")
    o_t = nc.dram_tensor("o_t", [P, NT], mybir.dt.float32, kind="ExternalOutput")

    with TileContext(nc) as tc:
        with (
            tc.tile_pool(name="work", bufs=2) as pool,
            tc.tile_pool(name="stats", bufs=1) as spool,
            tc.tile_pool(name="ps", bufs=1, space="PSUM") as psum,
        ):
            s_pp = spool.tile([P, NT], mybir.dt.float32)
            s_t = spool.tile([P, NT], mybir.dt.float32)
            junk = spool.tile([P, F], mybir.dt.float32, tag="junk")
            ones = spool.tile([P, 1], mybir.dt.float32, tag="ones")
            ps_pt = psum.tile([1, MMN], mybir.dt.float32, tag="ps_pt")
            ps_ppt = psum.tile([1, MMN], mybir.dt.float32, tag="ps_ppt")
            nc.any.memset(ones, 1.0)

            for i in range(NT):
                xt = pool.tile([P, F], mybir.dt.float32, tag="xt")
                tt = pool.tile([P, F], mybir.dt.int32, tag="tt")
                pt_ = pool.tile([P, F], mybir.dt.float32, tag="p")
                u = pool.tile([P, F], mybir.dt.float32, tag="u")
                w = pool.tile([P, F], mybir.dt.float32, tag="w")

                nc.sync.dma_start(xt[:], x[:, i * F : (i + 1) * F])
                nc.sync.dma_start(tt[:], t[:, i * F : (i + 1) * F])

                # p = sigmoid(x)                                   [ACT]
                nc.scalar.activation(
                    pt_[:], xt[:], mybir.ActivationFunctionType.Sigmoid
                )
                # sq = p² (reusing dead xt buffer), S_pp row-accum  [ACT]
                nc.scalar.activation(
                    xt[:],
                    pt_[:],
                    mybir.ActivationFunctionType.Square,
                    accum_out=s_pp[:, i : i + 1],
                )
                # u = p·t, w = p²·t  (int32 in1 converts in-pipe)   [DVE]
                nc.vector.tensor_tensor(
                    out=u[:], in0=pt_[:], in1=tt[:], op=mybir.AluOpType.mult
                )
                nc.vector.tensor_tensor(
                    out=w[:], in0=xt[:], in1=tt[:], op=mybir.AluOpType.mult
                )
                # S_t row-accum via float(t) copy                  [ACT]
                nc.scalar.activation(
                    junk[:],
                    tt[:],
                    mybir.ActivationFunctionType.Copy,
                    accum_out=s_t[:, i : i + 1],
                )
                # column-sum accumulation of u and w               [PE]
                for j in range(NCH):
                    nc.tensor.matmul(
                        ps_pt[:],
                        ones[:],
                        u[:, j * MMN : (j + 1) * MMN],
                        start=(i == 0 and j == 0),
                        stop=(i == NT - 1 and j == NCH - 1),
                    )
                for j in range(NCH):
                    nc.tensor.matmul(
                        ps_ppt[:],
                        ones[:],
                        w[:, j * MMN : (j + 1) * MMN],
                        start=(i == 0 and j == 0),
                        stop=(i == NT - 1 and j == NCH - 1),
                    )

            fin_pt = spool.tile([1, MMN], mybir.dt.float32, tag="fin_pt")
            fin_ppt = spool.tile([1, MMN], mybir.dt.float32, tag="fin_ppt")
            nc.vector.tensor_copy(fin_pt[:], ps_pt[:])
            nc.vector.tensor_copy(fin_ppt[:], ps_ppt[:])
            nc.sync.dma_start(o_pt[:], fin_pt[:])
            nc.sync.dma_start(o_ppt[:], fin_ppt[:])
            nc.sync.dma_start(o_pp[:], s_pp[:])
            nc.sync.dma_start(o_t[:], s_t[:])
    nc.finalize()
    return nc


def _get_nc():
    global _nc_cache
    if _nc_cache is None:
        _nc_cache = _build_bass()
    return _nc_cache


def kernel(input, target, _trace=False):
    x = np.ascontiguousarray(np.asarray(input, dtype=np.float32)).reshape(
        N_CORES, P, FREE
    )
    t = np.ascontiguousarray(np.asarray(target, dtype=np.int32)).reshape(
        N_CORES, P, FREE
    )
    in_maps = [{"input": x[i], "target": t[i]} for i in range(N_CORES)]

    nc = _get_nc()
    res = run_bass_kernel_spmd(
        nc, in_maps, core_ids=list(range(N_CORES)), trace=_trace
    )
    kernel.last_results = res

    s_pt = s_ppt = s_pp = s_t = 0.0
    for r in res.results:
        s_pt += float(r["o_pt"].astype(np.float64).sum())
        s_ppt += float(r["o_ppt"].astype(np.float64).sum())
        s_pp += float(r["o_pp"].astype(np.float64).sum())
        s_t += float(r["o_t"].astype(np.float64).sum())

    c1 = float(s_t)
    c0 = float(TOTAL - s_t)
    w0 = 1.0 / (c0 + SMOOTH) ** 2
    w1 = 1.0 / (c1 + SMOOTH) ** 2
    intersection = w1 * s_pt
    denominator = w0 * (s_pp - s_ppt) + w1 * (s_ppt + c1)
    dice = 1.0 - (2.0 * intersection + SMOOTH) / (denominator + SMOOTH)
    return np.asarray(dice, dtype=np.float32)


# revision 12
# speedup vs baseline: 1.2503x; 1.2503x over previous
"""Balanced dice loss (histogram binning) on 8 Trainium2 NeuronCores.

Math: with t ∈ {0,1} and p = sigmoid(x), the reference loss only needs
four global sums:
    S_t   = Σ t            (count of ones — the bincount)
    S_pt  = Σ p·t
    S_pp  = Σ p²
    S_ppt = Σ p²·t
Then with c1 = S_t, c0 = N − c1, w0 = 1/(c0+s)², w1 = 1/(c1+s)²:
    intersection = w1·S_pt
    denominator  = w0·(S_pp − S_ppt) + w1·(S_ppt + c1)
    dice = 1 − (2·I + s)/(D + s)

Device kernel (data-parallel over 8 cores, batch-sharded), per [128,F] tile:
    ACT : p = sigmoid(x); sq = p² (+row-accum → S_pp); float(t) (+accum → S_t)
    DVE : u = p·t, w = sq·t  (tensor_tensor with int32 in1, f32 out)
    PE  : ones[128,1]ᵀ @ u/w chunks → PSUM column-sum accumulation (S_pt, S_ppt)
Per-partition/per-tile partials are DMA'd out; host reduces in float64.
"""

import numpy as np

import concourse.bacc as bacc
import concourse.mybir as mybir
from concourse.bass_utils import run_bass_kernel_spmd
from concourse.tile import TileContext

N_CORES = 8
P = 128
TOTAL = 32 * 1024 * 1024  # elements in the full problem
PER_CORE = TOTAL // N_CORES  # 4,194,304
FREE = PER_CORE // P  # 32,768 f32 per partition
F = 4096  # tile free-dim
NT = FREE // F  # tiles per core
MMN = 512  # matmul moving free-dim (one PSUM bank)
NCH = F // MMN  # matmul chunks per tile
SMOOTH = 1e-05

_nc_cache = None


def _build_bass():
    nc = bacc.Bacc(None, target_bir_lowering=False)
    x = nc.dram_tensor("input", [P, FREE], mybir.dt.float32, kind="ExternalInput")
    t = nc.dram_tensor("target", [P, FREE], mybir.dt.int32, kind="ExternalInput")
    o_pt = nc.dram_tensor("o_pt", [1, MMN], mybir.dt.float32, kind="ExternalOutput")
    o_ppt = nc.dram_tensor("o_ppt", [1, MMN], mybir.dt.float32, kind="ExternalOutput")
    o_pp = nc.dram_tensor("o_pp", [P, NT], mybir.dt.float32, kind="ExternalOutput")
    o_t = nc.dram_tensor("o_t", [P, NT], mybir.dt.float32, kind="ExternalOutput")

    with TileContext(nc) as tc:
        with (
            tc.tile_pool(name="work", bufs=2) as pool,
            tc.tile_pool(name="stats", bufs=1) as spool,
            tc.tile_pool(name="ps", bufs=1, space="PSUM") as psum,
        ):
            s_pp = spool.tile([P, NT], mybir.dt.float32)
            s_t = spool.tile([P, NT], mybir.dt.float32)
            junk = spool.tile([P, F], mybir.dt.float32, tag="junk")
            ones = spool.tile([P, 1], mybir.dt.bfloat16, tag="ones")
            ps_pt = psum.tile([1, MMN], mybir.dt.float32, tag="ps_pt")
            ps_ppt = psum.tile([1, MMN], mybir.dt.float32, tag="ps_ppt")
            nc.any.memset(ones, 1.0)

            for i in range(NT):
                xt = pool.tile([P, F], mybir.dt.float32, tag="xt", bufs=3)
                tt = pool.tile([P, F], mybir.dt.int32, tag="tt", bufs=3)
                pt_ = pool.tile([P, F], mybir.dt.float32, tag="p")
                u = pool.tile([P, F], mybir.dt.bfloat16, tag="u")
                w = pool.tile([P, F], mybir.dt.bfloat16, tag="w")

                nc.sync.dma_start(xt[:], x[:, i * F : (i + 1) * F])
                nc.sync.dma_start(tt[:], t[:, i * F : (i + 1) * F])

                # p = sigmoid(x)                                   [ACT]
                nc.scalar.activation(
                    pt_[:], xt[:], mybir.ActivationFunctionType.Sigmoid
                )
                # sq = p² (reusing dead xt buffer), S_pp row-accum  [ACT]
                nc.scalar.activation(
                    xt[:],
                    pt_[:],
                    mybir.ActivationFunctionType.Square,
                    accum_out=s_pp[:, i : i + 1],
                )
                # u = p·t, w = p²·t  (int32 in1 converts in-pipe)   [DVE]
                nc.vector.tensor_tensor(
                    out=u[:], in0=pt_[:], in1=tt[:], op=mybir.AluOpType.mult
                )
                nc.vector.tensor_tensor(
                    out=w[:], in0=xt[:], in1=tt[:], op=mybir.AluOpType.mult
                )
                # S_t row-accum via float(t) copy                  [ACT]
                nc.scalar.activation(
                    junk[:],
                    tt[:],
                    mybir.ActivationFunctionType.Copy,
                    accum_out=s_t[:, i : i + 1],
                )
                # column-sum accumulation of u and w               [PE]
                for j in range(NCH):
                    nc.tensor.matmul(
                        ps_pt[:],
                        ones[:],
                        u[:, j * MMN : (j + 1) * MMN],
                        start=(i == 0 and j == 0),
                        stop=(i == NT - 1 and j == NCH - 1),
                    )
                for j in range(NCH):
                    nc.tensor.matmul(
                        ps_ppt[:],
                        ones[:],
                        w[:, j * MMN : (j + 1) * MMN],
                        start=(i == 0 and j == 0),
                        stop=(i == NT - 1 and j == NCH - 1),
                    )

            fin_pt = spool.tile([1, MMN], mybir.dt.float32, tag="fin_pt")
            fin_ppt = spool.tile([1, MMN], mybir.dt.float32, tag="fin_ppt")
            nc.vector.tensor_copy(fin_pt[:], ps_pt[:])
            nc.vector.tensor_copy(fin_ppt[:], ps_ppt[:])
            nc.sync.dma_start(o_pt[:], fin_pt[:])
            nc.sync.dma_start(o_ppt[:], fin_ppt[:])
            nc.sync.dma_start(o_pp[:], s_pp[:])
            nc.sync.dma_start(o_t[:], s_t[:])
    nc.finalize()
    return nc


def _get_nc():
    global _nc_cache
    if _nc_cache is None:
        _nc_cache = _build_bass()
    return _nc_cache


def kernel(input, target, _trace=False):
    x = np.ascontiguousarray(np.asarray(input, dtype=np.float32)).reshape(
        N_CORES, P, FREE
    )
    t = np.ascontiguousarray(np.asarray(target, dtype=np.int32)).reshape(
        N_CORES, P, FREE
    )
    in_maps = [{"input": x[i], "target": t[i]} for i in range(N_CORES)]

    nc = _get_nc()
    res = run_bass_kernel_spmd(
        nc, in_maps, core_ids=list(range(N_CORES)), trace=_trace
    )
    kernel.last_results = res

    s_pt = s_ppt = s_pp = s_t = 0.0
    for r in res.results:
        s_pt += float(r["o_pt"].astype(np.float64).sum())
        s_ppt += float(r["o_ppt"].astype(np.float64).sum())
        s_pp += float(r["o_pp"].astype(np.float64).sum())
        s_t += float(r["o_t"].astype(np.float64).sum())

    c1 = float(s_t)
    c0 = float(TOTAL - s_t)
    w0 = 1.0 / (c0 + SMOOTH) ** 2
    w1 = 1.0 / (c1 + SMOOTH) ** 2
    intersection = w1 * s_pt
    denominator = w0 * (s_pp - s_ppt) + w1 * (s_ppt + c1)
    dice = 1.0 - (2.0 * intersection + SMOOTH) / (denominator + SMOOTH)
    return np.asarray(dice, dtype=np.float32)
